# revision 1
# baseline (speedup 1.0000x reference)
# Trainium2 Bass kernel for CrossAttentionFusion.
#
# Reference computation (per batch b):
#   pet_seq = pet_feat[b] viewed as (C, L)^T            L = H*W = 4096, C = 512
#   q = pet_seq @ Wq.T ; k = ct_seq @ Wk.T ; v = ct_seq @ Wv.T   (8 heads, hd=64)
#   x = softmax(q k^T / sqrt(hd)) v                      per head
#   y = LN(pet_seq + x @ Wp.T + bp) * gamma + beta       -> (B, C, H, W)
#
# Sharding: 8 cores = 2 batches x 4 query-row chunks (1024 rows each).
# Each core computes K/V for its batch (replicated within the batch group),
# attention + output projection + LayerNorm for its 1024 query rows.
# Outputs are disjoint (C, 1024) column blocks of the final (B, C, L) tensor,
# so no collectives are needed.
#
# Everything on-device runs in "transposed" layout (channels on SBUF
# partitions, tokens on the free axis), which matches the (B, C, H, W) input
# and output layouts exactly:
#   QT/KT = W @ petT/ctT          scores^T = KT_h^T.T @ QT_h  (softmax dim on
#   partitions, summed via a ones-column appended to V)       O^T = V_aug^T.T @ P^T
#   y^T = WpT.T @ xT              LN stats via ones-matmuls, broadcast via
#   K=1 ones-matmuls.
#
# exp() is split between ScalarE (table exp, head A of each pair) and the
# vector engine (Schraudolph bit-trick exp emitting the top 16 bits of the
# fp32 pattern as int16, bitcast to bf16 — head B), because ScalarE alone
# (1 elem/cycle/lane) would be the kernel bottleneck.
#
# HW gotchas encoded here (CoreSim does not model them):
#  - DVE/ACT ops must have in/out APs at the SAME base partition; any
#    cross-partition move goes through DMA (or a ones-matmul broadcast).
#  - custom DVE ops (reciprocal_approx_*) read the wrong partition via this
#    compile path; only standard ISA ops are used.
#  - a tensor feeding a float32r matmul must be *written* as float32r
#    (walrus verifier rejects plain-f32 producers and f32/bf16 mixing).

import numpy as np
import ml_dtypes

import concourse.bacc as bacc
import concourse.bass as bass
import concourse.tile as tile
from concourse import mybir
from concourse import bass_utils
from concourse.alu_op_type import AluOpType
from contextlib import ExitStack

F32 = mybir.dt.float32
F32R = mybir.dt.float32r
BF16 = mybir.dt.bfloat16
I32 = mybir.dt.int32
I16 = mybir.dt.int16

B, C, H, W = 2, 512, 64, 64
L = H * W                    # 4096
NH, HD = 8, 64
NCORES = 8
LQ = L // 4                  # 1024 query rows per core
ATT_SCALE = HD ** -0.5       # 1/8
LN_EPS = 1e-5

# Schraudolph exp: exp(x) ~= bitcast_f32(int32(x*EXP_A + EXP_B)).  We emit the
# top 16 bits directly (int16 -> bitcast bf16), so the affine runs at 1/2^16
# scale; the result is a bf16-precision exp.
EXP_A = float(2 ** 23 / np.log(2.0))
EXP_B = float(127 * 2 ** 23 - 366400.0)
EXP_A16 = EXP_A / 65536.0
EXP_B16 = EXP_B / 65536.0

# Which m-chunks use ScalarE exp (vs DVE Schraudolph), interleaved
ACT_OF_8 = 4
S_BUFS = 2      # double-buffering depth for score psum tiles
PT_BUFS = 3     # buffering for exp output tiles


def build_nc(debug: bool = False, debug_taps: bool = False, repeat: int = 1):
    nc = bacc.Bacc("TRN2", target_bir_lowering=False, debug=debug,
                   num_devices=NCORES)

    # ---- DRAM I/O ----------------------------------------------------------
    pet_f = nc.dram_tensor("pet_t_f32", [C, LQ], F32, kind="ExternalInput").ap()
    pet_b = nc.dram_tensor("pet_t_bf16", [C, LQ], BF16, kind="ExternalInput").ap()
    ct_b = nc.dram_tensor("ct_t_bf16", [C, L], BF16, kind="ExternalInput").ap()
    wq_d = nc.dram_tensor("wq_t", [C, C], BF16, kind="ExternalInput").ap()
    wk_d = nc.dram_tensor("wk_t", [C, C], BF16, kind="ExternalInput").ap()
    wv_d = nc.dram_tensor("wv_t", [C, C], BF16, kind="ExternalInput").ap()
    wp_d = nc.dram_tensor("wp_t", [C, C], BF16, kind="ExternalInput").ap()
    gamma_d = nc.dram_tensor("gamma", [C, 1], F32, kind="ExternalInput").ap()
    beta_d = nc.dram_tensor("beta", [C, 1], F32, kind="ExternalInput").ap()
    bp_d = nc.dram_tensor("bp", [C, 1], F32, kind="ExternalInput").ap()
    out_d = nc.dram_tensor("out", [C, LQ], F32, kind="ExternalOutput").ap()
    taps = {}
    if debug_taps:
        taps["qt"] = nc.dram_tensor("dbg_qt", [C, LQ], BF16, kind="ExternalOutput").ap()
        taps["kt"] = nc.dram_tensor("dbg_kt", [C, L], BF16, kind="ExternalOutput").ap()
        taps["vt"] = nc.dram_tensor("dbg_vt", [L, NH * (HD + 1)], BF16, kind="ExternalOutput").ap()
        taps["xt"] = nc.dram_tensor("dbg_xt", [C, LQ], BF16, kind="ExternalOutput").ap()
        taps["xres"] = nc.dram_tensor("dbg_xres", [C, LQ], F32, kind="ExternalOutput").ap()
        taps["s0"] = nc.dram_tensor("dbg_s0", [128, 512], F32, kind="ExternalOutput").ap()
        taps["p_act"] = nc.dram_tensor("dbg_p_act", [128, 512], BF16, kind="ExternalOutput").ap()
        taps["p_dve"] = nc.dram_tensor("dbg_p_dve", [128, 512], BF16, kind="ExternalOutput").ap()
        taps["o0"] = nc.dram_tensor("dbg_o0", [HD + 1, 512], F32, kind="ExternalOutput").ap()
        taps["r0"] = nc.dram_tensor("dbg_r0", [1, 512], F32, kind="ExternalOutput").ap()

    NC4 = C // 128           # 4 chunks of 128 channels

    with tile.TileContext(nc) as tc, ExitStack() as top:
        persist = top.enter_context(tc.tile_pool(name="persist", bufs=1))

        # ---- resident tensors (sliced into 128-partition tiles) ------------
        # Allocated up front; DMAs for the phase-3/4-only tensors are emitted
        # inside phase 1 AFTER the projection-critical loads (wq/pet16/wk/ct/
        # wv, which live in a phase-1-scoped pool) so the PE starts sooner.
        def alloc(shape, dt, tag):
            return persist.tile(shape, dt, tag=tag, name=tag)

        pet32 = [alloc([128, LQ], F32, f"pet32_{i}") for i in range(NC4)]
        wp = [alloc([128, C], BF16, f"wp_{i}") for i in range(NC4)]
        gamma = [alloc([128, 1], F32, f"g_{i}") for i in range(NC4)]
        beta = [alloc([128, 1], F32, f"b_{i}") for i in range(NC4)]
        bp = [alloc([128, 1], F32, f"bp_{i}") for i in range(NC4)]

        qt = [persist.tile([128, LQ], BF16, tag=f"qt_{i}", name=f"qt_{i}") for i in range(NC4)]
        kt = [persist.tile([128, L], BF16, tag=f"kt_{i}", name=f"kt_{i}") for i in range(NC4)]
        # V rows, 65 cols per head (64 dims + ones column for the softmax sum)
        vt = [persist.tile([128, NH * (HD + 1)], BF16, tag=f"vt_{i}", name=f"vt_{i}")
              for i in range(L // 128)]
        xt = [persist.tile([128, LQ], BF16, tag=f"xt_{i}", name=f"xt_{i}") for i in range(NC4)]
        xres = [persist.tile([128, LQ], F32R, tag=f"xr_{i}", name=f"xr_{i}") for i in range(NC4)]

        # ones used as matmul lhsT for partition reductions / broadcasts
        # (memset cannot write f32r directly; write f32 then copy-round)
        ones_r = persist.tile([1, 128], F32R, tag="ones_r", name="ones_r")      # K=1 broadcast
        ones_c = persist.tile([128, 1], F32R, tag="ones_c", name="ones_c")      # partition sum
        ones_rf = persist.tile([1, 128], F32, tag="ones_rf", name="ones_rf")
        ones_cf = persist.tile([128, 1], F32, tag="ones_cf", name="ones_cf")
        nc.vector.memset(ones_rf[:], 1.0)
        nc.vector.memset(ones_cf[:], 1.0)
        nc.vector.tensor_copy(ones_r[:], ones_rf[:])
        nc.vector.tensor_copy(ones_c[:], ones_cf[:])

        for _rep in range(max(1, repeat)):
            # ---- phase 1: projections ------------------------------------------
            with tc.tile_pool(name="ph1", bufs=1) as ph1, \
                 tc.tile_pool(name="pj", bufs=4, space="PSUM") as pj:
                def p1load(ap_dram, shape, dt, tag):
                    t = ph1.tile(shape, dt, tag=tag, name=tag)
                    nc.sync.dma_start(t[:], ap_dram)
                    return t
                wq = [p1load(wq_d[i * 128:(i + 1) * 128, :], [128, C], BF16,
                             f"wq_{i}") for i in range(NC4)]
                pet16 = [p1load(pet_b[i * 128:(i + 1) * 128, :], [128, LQ],
                                BF16, f"pet16_{i}") for i in range(NC4)]
                wk = [p1load(wk_d[i * 128:(i + 1) * 128, :], [128, C], BF16,
                             f"wk_{i}") for i in range(NC4)]
                ct = [p1load(ct_b[i * 128:(i + 1) * 128, :], [128, L], BF16,
                             f"ct_{i}") for i in range(NC4)]
                wv = [p1load(wv_d[i * 128:(i + 1) * 128, :], [128, C], BF16,
                             f"wv_{i}") for i in range(NC4)]
                if _rep == 0:
                    for i in range(NC4):
                        nc.sync.dma_start(pet32[i][:], pet_f[i * 128:(i + 1) * 128, :])
                        nc.sync.dma_start(wp[i][:], wp_d[i * 128:(i + 1) * 128, :])
                        nc.sync.dma_start(gamma[i][:], gamma_d[i * 128:(i + 1) * 128, :])
                        nc.sync.dma_start(beta[i][:], beta_d[i * 128:(i + 1) * 128, :])
                        nc.sync.dma_start(bp[i][:], bp_d[i * 128:(i + 1) * 128, :])

                # QT[it] (128 chans, LQ) = sum_c WqT[c, it].T @ petT[c]
                for it in range(NC4):
                    for lc in range(LQ // 512):
                        ps = pj.tile([128, 512], F32, tag="pj", name="pj")
                        for c in range(NC4):
                            nc.tensor.matmul(
                                ps[:], wq[c][:, it * 128:(it + 1) * 128],
                                pet16[c][:, lc * 512:(lc + 1) * 512],
                                start=(c == 0), stop=(c == NC4 - 1))
                        nc.scalar.copy(qt[it][:, lc * 512:(lc + 1) * 512], ps[:])

                # KT[it] (128 chans, L)
                for it in range(NC4):
                    for mc in range(L // 512):
                        ps = pj.tile([128, 512], F32, tag="pj", name="pj")
                        for c in range(NC4):
                            nc.tensor.matmul(
                                ps[:], wk[c][:, it * 128:(it + 1) * 128],
                                ct[c][:, mc * 512:(mc + 1) * 512],
                                start=(c == 0), stop=(c == NC4 - 1))
                        nc.scalar.copy(kt[it][:, mc * 512:(mc + 1) * 512], ps[:])

                # V[m-chunk] (128 rows, 512 chans) -> scattered into 65-col blocks
                for m in range(L // 128):
                    ps = pj.tile([128, 512], F32, tag="pj", name="pj")
                    for c in range(NC4):
                        nc.tensor.matmul(
                            ps[:], ct[c][:, m * 128:(m + 1) * 128], wv[c][:],
                            start=(c == 0), stop=(c == NC4 - 1))
                    dst = vt[m].rearrange("p (h d) -> p h d", h=NH)[:, :, 0:HD]
                    src = ps.rearrange("p (h d) -> p h d", h=NH)
                    nc.scalar.copy(dst, src)
                    nc.vector.memset(
                        vt[m].rearrange("p (h d) -> p h d", h=NH)[:, :, HD:HD + 1], 1.0)

            # ---- phases 2-4: attention + norm + out-proj + LayerNorm -----------
            # lc-outer loop; after an lc's attention finishes, its
            # post-processing is emitted as small chunks between the NEXT lc's
            # attention pairs so the PE hides the norm/proj/LN latency under
            # attention matmuls. All post-chunk PSUM shares one 2-buffer "pp"
            # pool (broadcasts, proj, stats, LN apply rotate through it), which
            # keeps total PSUM at 8 banks: sA(2) sB(2) oA oB pp(2).
            NM = L // 128            # 32 m-chunks
            NLQ = LQ // 512          # 2 lq-chunks
            with tc.tile_pool(name="osb", bufs=1) as osbp, \
                 tc.tile_pool(name="ps_s", bufs=1, space="PSUM") as ps_s, \
                 tc.tile_pool(name="ps_o", bufs=1, space="PSUM") as ps_o, \
                 tc.tile_pool(name="pt", bufs=1) as ptp, \
                 tc.tile_pool(name="pp", bufs=2, space="PSUM") as pp, \
                 tc.tile_pool(name="nrm", bufs=2) as nrm, \
                 tc.tile_pool(name="tmp", bufs=2) as tmp, \
                 tc.tile_pool(name="lrows", bufs=1) as lrows, \
                 tc.tile_pool(name="yout", bufs=2) as yout:
                stores = {}          # (pair, lc, slot) -> o_sb tile

                def attention(pair, lc):
                    hA, hB = 2 * pair, 2 * pair + 1
                    oA = ps_o.tile([HD + 1, 512], F32, tag="oA", name="oA")
                    oB = ps_o.tile([HD + 1, 512], F32, tag="oB", name="oB")
                    for m in range(NM):
                        sA = ps_s.tile([128, 512], F32, tag="sA", bufs=2, name="sA")
                        sB = ps_s.tile([128, 512], F32, tag="sB", bufs=2, name="sB")
                        # packed head-pair scores: S^T[m, lq] (K = 64)
                        nc.tensor.matmul(
                            sA[:], kt[pair][0:64, m * 128:(m + 1) * 128],
                            qt[pair][0:64, lc * 512:(lc + 1) * 512])
                        nc.tensor.matmul(
                            sB[:], kt[pair][64:128, m * 128:(m + 1) * 128],
                            qt[pair][64:128, lc * 512:(lc + 1) * 512])
                        rhs = []
                        for s, nm in ((sA, "A"), (sB, "B")):
                            use_act = (nm == "A") if ACT_OF_8 == 4 else \
                                ((m * ACT_OF_8) % 8 < ACT_OF_8)
                            if use_act:
                                p = ptp.tile([128, 512], BF16, tag=f"pt{nm}_bf",
                                             bufs=PT_BUFS, name=f"pt{nm}_bf")
                                nc.scalar.activation(
                                    p[:], s[:],
                                    mybir.ActivationFunctionType.Exp,
                                    scale=ATT_SCALE)
                                rhs.append(p[:])
                            else:
                                p = ptp.tile([128, 512], I16, tag=f"pt{nm}_i",
                                             bufs=PT_BUFS, name=f"pt{nm}_i")
                                nc.vector.tensor_scalar(
                                    p[:], s[:], EXP_A16 * ATT_SCALE, EXP_B16,
                                    AluOpType.mult, AluOpType.add)
                                rhs.append(p[:].bitcast(BF16))
                        nc.tensor.matmul(oA[:], vt[m][:, hA * 65:hA * 65 + 65],
                                         rhs[0], start=(m == 0), stop=(m == NM - 1))
                        nc.tensor.matmul(oB[:], vt[m][:, hB * 65:hB * 65 + 65],
                                         rhs[1], start=(m == 0), stop=(m == NM - 1))
                    for o, slot in ((oA, 0), (oB, 1)):
                        o_sb = osbp.tile([HD + 1, 512], F32,
                                         tag=f"osb_{pair}_{lc}_{slot}",
                                         name=f"osb_{pair}_{lc}_{slot}")
                        nc.vector.tensor_copy(o_sb[:], o[:])
                        stores[(pair, lc, slot)] = o_sb

                def norm_chunk(lc):
                    # Batched softmax denominators -> one reciprocal per lc.
                    # DVE lanes cannot move data across partitions on HW, so
                    # every partition-moving step is a DMA (row gathers,
                    # broadcast-row staging, odd head slots to partitions
                    # 64-127).
                    ents = [(p, s) for p in range(NH // 2) for s in (0, 1)]
                    den = osbp.tile([len(ents), 512], F32, tag="den",
                                    name=f"den{lc}")
                    for i, (p, s) in enumerate(ents):
                        nc.sync.dma_start(den[i:i + 1, :],
                                          stores[(p, lc, s)][64:65, :])
                    nc.vector.reciprocal(den[:], den[:])
                    rec_r = osbp.tile([len(ents), 512], F32R, tag="recr",
                                      name=f"recr{lc}")
                    nc.vector.tensor_copy(rec_r[:], den[:])
                    sl = slice(lc * 512, (lc + 1) * 512)
                    for i, (pair, slot) in enumerate(ents):
                        o_sb = stores[(pair, lc, slot)]
                        rr = nrm.tile([1, 512], F32R, tag="rr", name="rr")
                        nc.sync.dma_start(rr[:], rec_r[i:i + 1, :])
                        bc = pp.tile([128, 512], F32, tag="pp", name="bcn")
                        nc.tensor.matmul(bc[0:64, :], ones_r[:, 0:64], rr[:])
                        if slot == 0:
                            nc.vector.tensor_tensor(
                                xt[pair][0:64, sl], o_sb[0:64, :], bc[0:64, :],
                                AluOpType.mult)
                        else:
                            xb = nrm.tile([64, 512], BF16, tag="xb", name="xb")
                            nc.vector.tensor_tensor(xb[:], o_sb[0:64, :],
                                                    bc[0:64, :], AluOpType.mult)
                            nc.sync.dma_start(xt[pair][64:128, sl], xb[:])

                def proj_chunk(lc):
                    sl = slice(lc * 512, (lc + 1) * 512)
                    for it in range(NC4):
                        ps = pp.tile([128, 512], F32, tag="pp", name="psy")
                        for c in range(NC4):
                            nc.tensor.matmul(ps[:], wp[c][:, it * 128:(it + 1) * 128],
                                             xt[c][:, sl],
                                             start=(c == 0), stop=(c == NC4 - 1))
                        # xres = (y + bp) + petT
                        nc.vector.scalar_tensor_tensor(
                            xres[it][:, sl], ps[:], bp[it][:], pet32[it][:, sl],
                            AluOpType.add, AluOpType.add)

                stats = {}

                def ln_stats_chunk(lc):
                    sl = slice(lc * 512, (lc + 1) * 512)
                    psum = pp.tile([128, 512], F32, tag="pp", name="psum_sum")
                    for c in range(NC4):
                        nc.tensor.matmul(psum[0:1, :], ones_c[:], xres[c][:, sl],
                                         start=(c == 0), stop=(c == NC4 - 1))
                    psq = pp.tile([128, 512], F32, tag="pp", name="psum_sq")
                    for c in range(NC4):
                        xsq = tmp.tile([128, 512], F32R, tag="xsq", name="xsq")
                        nc.vector.tensor_tensor(xsq[:], xres[c][:, sl],
                                                xres[c][:, sl], AluOpType.mult)
                        nc.tensor.matmul(psq[0:1, :], ones_c[:], xsq[:],
                                         start=(c == 0), stop=(c == NC4 - 1))
                    mu = lrows.tile([1, 512], F32R, tag=f"mu{lc}", name=f"mu{lc}")
                    ve = lrows.tile([1, 512], F32, tag="ve", name=f"ve{lc}")
                    t0 = lrows.tile([1, 512], F32, tag="t0", name=f"t0{lc}")
                    rstd = lrows.tile([1, 512], F32R, tag=f"rs{lc}", name=f"rs{lc}")
                    nc.vector.tensor_scalar(mu[:], psum[0:1, :], 1.0 / C, None,
                                            AluOpType.mult)
                    nc.vector.tensor_tensor(t0[:], mu[:], mu[:], AluOpType.mult)
                    # ve = sumsq/C + eps - mu^2
                    nc.vector.scalar_tensor_tensor(ve[:], psq[0:1, :], 1.0 / C,
                                                   t0[:], AluOpType.mult,
                                                   AluOpType.subtract)
                    nc.vector.tensor_scalar(ve[:], ve[:], LN_EPS, None,
                                            AluOpType.add)
                    nc.scalar.activation(t0[:], ve[:],
                                         mybir.ActivationFunctionType.Sqrt)
                    r0 = lrows.tile([1, 512], F32, tag="r0", name=f"r0{lc}")
                    nc.vector.reciprocal(r0[:], t0[:])
                    # one Newton step: rstd = r0 * (1.5 - 0.5 * ve * r0^2)
                    nc.vector.tensor_tensor(t0[:], r0[:], r0[:], AluOpType.mult)
                    nc.vector.tensor_tensor(t0[:], t0[:], ve[:], AluOpType.mult)
                    nc.vector.tensor_scalar(t0[:], t0[:], -0.5, 1.5,
                                            AluOpType.mult, AluOpType.add)
                    nc.vector.tensor_tensor(rstd[:], r0[:], t0[:], AluOpType.mult)
                    stats[lc] = (mu, rstd)

                def ln_apply_chunk(lc):
                    sl = slice(lc * 512, (lc + 1) * 512)
                    mu, rstd = stats[lc]
                    bmu = pp.tile([128, 512], F32, tag="pp", name="bmu")
                    brs = pp.tile([128, 512], F32, tag="pp", name="brs")
                    nc.tensor.matmul(bmu[:], ones_r[:], mu[:])
                    nc.tensor.matmul(brs[:], ones_r[:], rstd[:])
                    for c in range(NC4):
                        t = tmp.tile([128, 512], F32, tag="lnt", bufs=1,
                                     name="lnt")
                        y = yout.tile([128, 512], F32, tag="y", name="yout")
                        nc.vector.tensor_tensor(t[:], xres[c][:, sl], bmu[:],
                                                AluOpType.subtract)
                        nc.vector.tensor_tensor(t[:], t[:], brs[:],
                                                AluOpType.mult)
                        nc.vector.tensor_scalar(y[:], t[:], gamma[c][:],
                                                beta[c][:], AluOpType.mult,
                                                AluOpType.add)
                        nc.sync.dma_start(out_d[c * 128:(c + 1) * 128, sl], y[:])

                chunks = []
                for lc in range(NLQ):
                    for pair in range(NH // 2):
                        attention(pair, lc)
                        if chunks:
                            chunks.pop(0)()
                    chunks += [lambda lc=lc: norm_chunk(lc),
                               lambda lc=lc: proj_chunk(lc),
                               lambda lc=lc: ln_stats_chunk(lc),
                               lambda lc=lc: ln_apply_chunk(lc)]
                while chunks:
                    chunks.pop(0)()

        if debug_taps:
            for i in range(NC4):
                nc.sync.dma_start(taps["qt"][i * 128:(i + 1) * 128, :], qt[i][:])
                nc.sync.dma_start(taps["kt"][i * 128:(i + 1) * 128, :], kt[i][:])
                nc.sync.dma_start(taps["xt"][i * 128:(i + 1) * 128, :], xt[i][:])
                nc.sync.dma_start(taps["xres"][i * 128:(i + 1) * 128, :],
                                  xres[i][:].bitcast(F32))
            for m in range(L // 128):
                nc.sync.dma_start(taps["vt"][m * 128:(m + 1) * 128, :], vt[m][:])

    nc.compile()
    return nc


def prep_core_inputs(inputs):
    """Shard + lay out the full inputs for the 8 cores."""
    pet = np.asarray(inputs["pet_feat"], np.float32).reshape(B, C, L)
    ct = np.asarray(inputs["ct_feat"], np.float32).reshape(B, C, L)
    bf = ml_dtypes.bfloat16
    wq_t = np.ascontiguousarray(np.asarray(inputs["Wq"], np.float32).T).astype(bf)
    wk_t = np.ascontiguousarray(np.asarray(inputs["Wk"], np.float32).T).astype(bf)
    wv_t = np.ascontiguousarray(np.asarray(inputs["Wv"], np.float32).T).astype(bf)
    wp_t = np.ascontiguousarray(np.asarray(inputs["Wp"], np.float32).T).astype(bf)
    gamma = np.asarray(inputs["gamma"], np.float32).reshape(C, 1)
    beta = np.asarray(inputs["beta"], np.float32).reshape(C, 1)
    bp = np.asarray(inputs["bp"], np.float32).reshape(C, 1)
    in_maps = []
    for core in range(NCORES):
        b, j = divmod(core, 4)
        sl = slice(j * LQ, (j + 1) * LQ)
        pet_sl = np.ascontiguousarray(pet[b][:, sl])
        in_maps.append({
            "pet_t_f32": pet_sl,
            "pet_t_bf16": pet_sl.astype(bf),
            "ct_t_bf16": np.ascontiguousarray(ct[b]).astype(bf),
            "wq_t": wq_t, "wk_t": wk_t, "wv_t": wv_t, "wp_t": wp_t,
            "gamma": gamma, "beta": beta, "bp": bp,
        })
    return in_maps


def assemble_output(results):
    out = np.empty((B, C, L), np.float32)
    for core in range(NCORES):
        b, j = divmod(core, 4)
        out[b][:, j * LQ:(j + 1) * LQ] = results[core]["out"]
    return out.reshape(B, C, H, W)


_NC_CACHE = {}


def get_nc(debug=False):
    key = debug
    if key not in _NC_CACHE:
        _NC_CACHE[key] = build_nc(debug=debug)
    return _NC_CACHE[key]


def kernel(**inputs):
    nc = get_nc()
    in_maps = prep_core_inputs(inputs)
    res = bass_utils.run_bass_kernel_spmd(nc, in_maps, list(range(NCORES)))
    return assemble_output(res.results)



# revision 18
# speedup vs baseline: 1.1722x; 1.1722x over previous
# Trainium2 Bass kernel for CrossAttentionFusion.
#
# Reference computation (per batch b):
#   pet_seq = pet_feat[b] viewed as (C, L)^T            L = H*W = 4096, C = 512
#   q = pet_seq @ Wq.T ; k = ct_seq @ Wk.T ; v = ct_seq @ Wv.T   (8 heads, hd=64)
#   x = softmax(q k^T / sqrt(hd)) v                      per head
#   y = LN(pet_seq + x @ Wp.T + bp) * gamma + beta       -> (B, C, H, W)
#
# Sharding: 8 cores = 2 batches x 4 query-row chunks (1024 rows each).
# Each core computes K/V for its batch (replicated within the batch group),
# attention + output projection + LayerNorm for its 1024 query rows.
# Outputs are disjoint (C, 1024) column blocks of the final (B, C, L) tensor,
# so no collectives are needed.
#
# Layouts:
#   Projections run transposed (channels on partitions): QT/KT = W @ petT/ctT.
#   Scores:   S^T[keys 128, queries 512] per head-pair (one [128,1024] PSUM
#             tile, half per head), K = 64.
#   exp:      the two head-halves of each m-chunk run on two DIFFERENT engines
#             in parallel, weighted-rotated across ACT (table exp) / DVE /
#             GPSIMD ("Pool") (Schraudolph bit-trick exp: int16 truncation of
#             the fp32 pattern -> bitcast bf16).
#   AV:       FLIPPED - P^T chunks are the stationary operand, V rows are
#             moving: o[q 128, 65] += P^T[:, qc].T @ V_aug[m].  Output
#             partitions = 128 queries (full PE output rate); col 64 is the
#             softmax denominator (ones column in V_aug).  The denominator is
#             per-PARTITION, so softmax normalization is a plain tensor_scalar
#             with a [128,1] reciprocal - no cross-partition moves.
#   x:        assembled in [q, c] layout, then DMA-transposed (XBAR) back to
#             [c, q] for the output projection; proj/LN as in v1.
#
# PSUM (8 banks): S double-buffered (2x2) + oA + oB + 2 rotating post banks.
# o-bank accumulation groups are opened once per pair with a 1-column zero
# matmul (start=True marks the whole 2KB zero region pending-zero), all AV
# matmuls accumulate with start=False, and the last one carries stop=True.
#
# HW gotchas encoded here (CoreSim does not model them):
#  - DVE/ACT ops must have in/out APs at the SAME base partition; any
#    cross-partition move goes through DMA (or a ones-matmul broadcast).
#  - custom DVE ops (reciprocal_approx_*) read the wrong partition via this
#    compile path; only standard ISA ops are used.
#  - a tensor feeding a float32r matmul must be *written* as float32r
#    (walrus verifier rejects plain-f32 producers and f32/bf16 mixing).

import numpy as np
import ml_dtypes

import concourse.bacc as bacc
import concourse.bass as bass
import concourse.tile as tile
from concourse import mybir
from concourse import bass_utils
from concourse.alu_op_type import AluOpType
from contextlib import ExitStack

F32 = mybir.dt.float32
F32R = mybir.dt.float32r
BF16 = mybir.dt.bfloat16
I32 = mybir.dt.int32
I16 = mybir.dt.int16

B, C, H, W = 2, 512, 64, 64
L = H * W                    # 4096
NH, HD = 8, 64
NCORES = 8
LQ = L // 4                  # 1024 query rows per core
ATT_SCALE = HD ** -0.5       # 1/8
LN_EPS = 1e-5

# Schraudolph exp: exp(x) ~= bitcast_f32(int32(x*EXP_A + EXP_B)).  We emit the
# top 16 bits directly (int16 -> bitcast bf16), so the affine runs at 1/2^16
# scale; the result is a bf16-precision exp.
EXP_A = float(2 ** 23 / np.log(2.0))
EXP_B = float(127 * 2 ** 23 - 366400.0)
EXP_A16 = EXP_A / 65536.0
EXP_B16 = EXP_B / 65536.0

S_BUFS = 3      # buffering depth for score psum tiles (per half)
PT_BUFS = 4     # buffering for exp output tiles


class EngineBalancer:
    """Weighted greedy assignment across the three element-wise engines.

    Costs are the cost-model ns for a [128, 512] op on each engine; pick()
    returns the engine with the least accumulated time, add()ing its cost.
    """

    def __init__(self):
        self.t = {"act": 0.0, "dve": 0.0, "pool": 0.0}

    COST = {"act": 570.0, "dve": 658.0, "pool": 806.0}

    def pick(self, exclude=(), scale=1.0):
        cands = [e for e in ("act", "dve", "pool") if e not in exclude]
        e = min(cands, key=lambda e: self.t[e] + self.COST[e] * scale)
        self.t[e] += self.COST[e] * scale
        return e

    def add(self, eng, scale=1.0):
        self.t[eng] += self.COST[eng] * scale


def build_nc(debug: bool = False, repeat: int = 1):
    nc = bacc.Bacc("TRN2", target_bir_lowering=False, debug=debug,
                   num_devices=NCORES)

    # ---- DRAM I/O ----------------------------------------------------------
    pet_f = nc.dram_tensor("pet_t_f32", [C, LQ], F32, kind="ExternalInput").ap()
    pet_b = nc.dram_tensor("pet_t_bf16", [C, LQ], BF16, kind="ExternalInput").ap()
    ct_b = nc.dram_tensor("ct_t_bf16", [C, L], BF16, kind="ExternalInput").ap()
    wq_d = nc.dram_tensor("wq_t", [C, C], BF16, kind="ExternalInput").ap()
    wk_d = nc.dram_tensor("wk_t", [C, C], BF16, kind="ExternalInput").ap()
    wv_d = nc.dram_tensor("wv_t", [C, C], BF16, kind="ExternalInput").ap()
    wp_d = nc.dram_tensor("wp_t", [C, C], BF16, kind="ExternalInput").ap()
    gamma_d = nc.dram_tensor("gamma", [C, 1], F32, kind="ExternalInput").ap()
    beta_d = nc.dram_tensor("beta", [C, 1], F32, kind="ExternalInput").ap()
    bp_d = nc.dram_tensor("bp", [C, 1], F32, kind="ExternalInput").ap()
    out_d = nc.dram_tensor("out", [C, LQ], F32, kind="ExternalOutput").ap()

    NC4 = C // 128           # 4 chunks of 128 channels
    NM = L // 128            # 32 key chunks
    NLQ = LQ // 512          # 2 lq-chunks
    NQC = 4                  # 128-query chunks per lq-chunk

    with tile.TileContext(nc) as tc, ExitStack() as top:
        persist = top.enter_context(tc.tile_pool(name="persist", bufs=1))

        def alloc(shape, dt, tag):
            return persist.tile(shape, dt, tag=tag, name=tag)

        # channel-chunked weights/stats live as single tiles with the chunk
        # index as a free dim, so each loads with ONE dma (fewer HWDGE slots)
        pet32_t = alloc([128, NC4, LQ], F32, "pet32")
        wp_t = alloc([128, NC4, C], BF16, "wp")
        gamma_t = alloc([128, NC4], F32, "gamma_t")
        beta_t = alloc([128, NC4], F32, "beta_t")
        bp_t = alloc([128, NC4], F32, "bp_t")
        pet32 = [pet32_t[:, i, :] for i in range(NC4)]
        wp = [wp_t[:, i, :] for i in range(NC4)]
        gamma = [gamma_t[:, i:i + 1] for i in range(NC4)]
        beta = [beta_t[:, i:i + 1] for i in range(NC4)]
        bp = [bp_t[:, i:i + 1] for i in range(NC4)]

        qt = [alloc([128, LQ], BF16, f"qt_{i}") for i in range(NC4)]
        kt = [alloc([128, L], BF16, f"kt_{i}") for i in range(NC4)]
        # V rows, 65 cols per head (64 dims + ones column for the softmax sum)
        vt = [alloc([128, NH * (HD + 1)], BF16, f"vt_{i}") for i in range(NM)]
        xres = [alloc([128, LQ], F32R, f"xr_{i}") for i in range(NC4)]

        # ones used as matmul lhsT for partition reductions / broadcasts
        # (memset cannot write f32r directly; write f32 then copy-round)
        ones_r = persist.tile([1, 128], F32R, tag="ones_r", name="ones_r")
        ones_c = persist.tile([128, 1], F32R, tag="ones_c", name="ones_c")
        ones_rf = persist.tile([1, 128], F32, tag="ones_rf", name="ones_rf")
        ones_cf = persist.tile([128, 1], F32, tag="ones_cf", name="ones_cf")
        nc.vector.memset(ones_rf[:], 1.0)
        nc.vector.memset(ones_cf[:], 1.0)
        nc.vector.tensor_copy(ones_r[:], ones_rf[:])
        nc.vector.tensor_copy(ones_c[:], ones_cf[:])
        # 1-element zero operands for opening PSUM accumulation groups
        zrow = persist.tile([1, 128], BF16, tag="zrow", name="zrow")
        zcol = persist.tile([1, 1], BF16, tag="zcol", name="zcol")
        nc.vector.memset(zrow[:], 0.0)
        nc.vector.memset(zcol[:], 0.0)

        bal = EngineBalancer()

        def ts_op(eng, dst, src, s1, s2, op1, op2=None):
            e = {"act": None, "dve": nc.vector, "pool": nc.gpsimd}[eng]
            if e is None:
                raise ValueError("act not valid for tensor_scalar here")
            if op2 is None:
                e.tensor_scalar(dst, src, s1, s2, op1)
            else:
                e.tensor_scalar(dst, src, s1, s2, op1, op2)

        def copy_tile(dst_ap, src_ap, scale_cols=1.0):
            # GPSIMD cannot touch PSUM on HW: copies are ACT/DVE only
            eng = bal.pick(exclude=("pool",), scale=scale_cols)
            if eng == "act":
                nc.scalar.copy(dst_ap, src_ap)
            else:
                nc.vector.tensor_copy(dst_ap, src_ap)

        for _rep in range(max(1, repeat)):
            # ---- phase 1: projections --------------------------------------
            with tc.tile_pool(name="ph1", bufs=1) as ph1, \
                 tc.tile_pool(name="pj", bufs=4, space="PSUM") as pj:
                def p1tile(shape, dt, tag):
                    return ph1.tile(shape, dt, tag=tag, name=tag)
                wq_s = p1tile([128, NC4, C], BF16, "wq")
                nc.sync.dma_start(wq_s[:], wq_d.rearrange("(c p) k -> p c k", p=128))
                pet16_s = p1tile([128, NC4, LQ], BF16, "pet16")
                nc.sync.dma_start(pet16_s[:], pet_b.rearrange("(c p) k -> p c k", p=128))
                wk_s = p1tile([128, NC4, C], BF16, "wk")
                nc.sync.dma_start(wk_s[:], wk_d.rearrange("(c p) k -> p c k", p=128))
                ct_s = p1tile([128, NC4, L], BF16, "ct")
                ct_r = ct_b.rearrange("(c p) k -> p c k", p=128)
                for u in range(4):
                    usl = slice(u * (L // 4), (u + 1) * (L // 4))
                    nc.sync.dma_start(ct_s[:, :, usl], ct_r[:, :, usl])
                wv_s = p1tile([128, NC4, C], BF16, "wv")
                nc.sync.dma_start(wv_s[:], wv_d.rearrange("(c p) k -> p c k", p=128))
                wq = [wq_s[:, i, :] for i in range(NC4)]
                pet16 = [pet16_s[:, i, :] for i in range(NC4)]
                wk = [wk_s[:, i, :] for i in range(NC4)]
                ct = [ct_s[:, i, :] for i in range(NC4)]
                wv = [wv_s[:, i, :] for i in range(NC4)]
                if _rep == 0:
                    nc.sync.dma_start(pet32_t[:],
                                      pet_f.rearrange("(c p) k -> p c k", p=128))
                    nc.sync.dma_start(wp_t[:],
                                      wp_d.rearrange("(c p) k -> p c k", p=128))
                    nc.sync.dma_start(gamma_t[:],
                                      gamma_d.rearrange("(c p) k -> p (c k)", p=128))
                    nc.sync.dma_start(beta_t[:],
                                      beta_d.rearrange("(c p) k -> p (c k)", p=128))
                    nc.sync.dma_start(bp_t[:],
                                      bp_d.rearrange("(c p) k -> p (c k)", p=128))

                # QT[it] (128 chans, LQ) = sum_c WqT[c, it].T @ petT[c]
                for it in range(NC4):
                    for lc in range(LQ // 512):
                        ps = pj.tile([128, 512], F32, tag="pj", name="pj")
                        for c in range(NC4):
                            nc.tensor.matmul(
                                ps[:], wq[c][:, it * 128:(it + 1) * 128],
                                pet16[c][:, lc * 512:(lc + 1) * 512],
                                start=(c == 0), stop=(c == NC4 - 1))
                        copy_tile(qt[it][:, lc * 512:(lc + 1) * 512], ps[:])

                # KT[it] (128 chans, L)
                for it in range(NC4):
                    for mc in range(L // 512):
                        ps = pj.tile([128, 512], F32, tag="pj", name="pj")
                        for c in range(NC4):
                            nc.tensor.matmul(
                                ps[:], wk[c][:, it * 128:(it + 1) * 128],
                                ct[c][:, mc * 512:(mc + 1) * 512],
                                start=(c == 0), stop=(c == NC4 - 1))
                        copy_tile(kt[it][:, mc * 512:(mc + 1) * 512], ps[:])

                # V[m-chunk] (128 rows, 512 chans) -> scattered into 65-col blocks
                for m in range(NM):
                    ps = pj.tile([128, 512], F32, tag="pj", name="pj")
                    for c in range(NC4):
                        nc.tensor.matmul(
                            ps[:], ct[c][:, m * 128:(m + 1) * 128], wv[c][:],
                            start=(c == 0), stop=(c == NC4 - 1))
                    dst = vt[m].rearrange("p (h d) -> p h d", h=NH)[:, :, 0:HD]
                    src = ps.rearrange("p (h d) -> p h d", h=NH)
                    copy_tile(dst, src)
                    nc.vector.memset(
                        vt[m].rearrange("p (h d) -> p h d", h=NH)[:, :, HD:HD + 1], 1.0)

            # ---- phases 2-4: attention + norm + out-proj + LayerNorm -------
            with tc.tile_pool(name="xp", bufs=2) as xp, \
                 tc.tile_pool(name="ps_s", bufs=1, space="PSUM") as ps_s, \
                 tc.tile_pool(name="ps_o", bufs=1, space="PSUM") as ps_o, \
                 tc.tile_pool(name="pt", bufs=1) as ptp, \
                 tc.tile_pool(name="nrm", bufs=2) as nrm, \
                 tc.tile_pool(name="tmp", bufs=2) as tmp, \
                 tc.tile_pool(name="lrows", bufs=1) as lrows, \
                 tc.tile_pool(name="yout", bufs=2) as yout:

                # post-processing PSUM tiles borrow buffers from the score
                # rotation (no separate pool - all 8 banks: SA*3 SB*3 oA oB)
                pp_state = [0]

                def pp_tile(name):
                    pp_state[0] += 1
                    t = ps_s.tile([128, 1024], F32, tag="S", bufs=S_BUFS,
                                  name=name)
                    return t

                def emit_exp(eng, dst, src):
                    # dst/src are [128, 512] bf16-out / f32-psum-in
                    if eng == "act":
                        nc.scalar.activation(
                            dst, src, mybir.ActivationFunctionType.Exp,
                            scale=ATT_SCALE)
                    elif eng == "dve":
                        nc.vector.tensor_scalar(
                            dst.bitcast(I16), src, EXP_A16 * ATT_SCALE,
                            EXP_B16, AluOpType.mult, AluOpType.add)
                    else:
                        nc.gpsimd.tensor_scalar(
                            dst.bitcast(I16), src, EXP_A16 * ATT_SCALE,
                            EXP_B16, AluOpType.mult, AluOpType.add)

                # exp alternates whole-m between ACT and DVE (GPSIMD cannot
                # read PSUM on real HW, so it cannot run exp).  One [128,1024]
                # instruction per m: ACT 996ns / DVE 1190ns, each under the
                # two-window budget with S_BUFS=3 + 3-m AV lookahead.
                exp_ctr = [0]

                def attention(pair, lc, x_lc):
                    hA, hB = 2 * pair, 2 * pair + 1
                    # [q 128, qc 4, 65] accumulators + 1 scratch col for the
                    # group opener; col 64 of each 65-block = softmax denom
                    oA_t = ps_o.tile([128, NQC * (HD + 1) + 1], F32, tag="oA",
                                     name="oA")
                    oB_t = ps_o.tile([128, NQC * (HD + 1) + 1], F32, tag="oB",
                                     name="oB")
                    oA = oA_t[:, 0:NQC * (HD + 1)].rearrange(
                        "p (q d) -> p q d", d=HD + 1)
                    oB = oB_t[:, 0:NQC * (HD + 1)].rearrange(
                        "p (q d) -> p q d", d=HD + 1)
                    sl = slice(lc * 512, (lc + 1) * 512)

                    def emit_av(m, pt, fin):
                        # ordered by exp-piece completion (ACT piece cols
                        # 0:384, DVE 384:768, POOL 768:1024) so the in-order
                        # PE wait queue drains progressively, without
                        # head-of-line inversions on the slower pieces.
                        for slot, (o, c0) in enumerate(
                                [(oA, 0), (oA, 128), (oA, 256),      # act
                                 (oA, 384), (oB, 512), (oB, 640),    # dve
                                 (oB, 768), (oB, 896)]):             # pool
                            h = hA if o is oA else hB
                            qc = (c0 % 512) // 128
                            f = fin and slot in (3, 7)
                            nc.tensor.matmul(
                                o[:, qc, :], pt[:, c0:c0 + 128],
                                vt[m][:, h * 65:h * 65 + 65],
                                start=False, stop=f)

                    pending = []
                    for m in range(NM):
                        S = ps_s.tile([128, 1024], F32, tag="S", bufs=S_BUFS,
                                      name="S")
                        msl = slice(m * 128, (m + 1) * 128)
                        nc.tensor.matmul(S[:, 0:512], kt[pair][0:64, msl],
                                         qt[pair][0:64, sl])
                        nc.tensor.matmul(S[:, 512:1024], kt[pair][64:128, msl],
                                         qt[pair][64:128, sl])
                        pt = ptp.tile([128, 1024], BF16, tag="pt",
                                      bufs=PT_BUFS, name="pt")
                        eng = ("act", "dve")[exp_ctr[0] % 2]
                        exp_ctr[0] += 1
                        emit_exp(eng, pt[:], S[:])
                        bal.add(eng, scale=2.0)
                        if m == 0:
                            # open the o accumulation groups (see header)
                            nc.tensor.matmul(oA_t[:, 260:261], zrow[:], zcol[:],
                                             start=True, stop=False)
                            nc.tensor.matmul(oB_t[:, 260:261], zrow[:], zcol[:],
                                             start=True, stop=False)
                        pending.append((m, pt))
                        # three-m lookahead: AV for m-3 runs while exp(m-2..m)
                        # are still in flight on the element-wise engines
                        if len(pending) > 3:
                            pm, ppt = pending.pop(0)
                            emit_av(pm, ppt, fin=False)
                    while pending:
                        pm, ppt = pending.pop(0)
                        emit_av(pm, ppt, fin=(pm == NM - 1))
                    # softmax normalize straight out of PSUM into x (q, c)
                    for h, o in ((hA, oA), (hB, oB)):
                        rec = nrm.tile([128, NQC], F32, tag="rec", name="rec")
                        nc.vector.reciprocal(rec[:], o[:, :, HD])
                        for qc in range(NQC):
                            dst = x_lc[:, qc, h * HD:(h + 1) * HD]
                            eng = bal.pick(exclude=("pool",), scale=0.25)
                            if eng == "act":
                                nc.scalar.activation(
                                    dst, o[:, qc, 0:HD],
                                    mybir.ActivationFunctionType.Copy,
                                    scale=rec[:, qc:qc + 1])
                            else:
                                ts_op(eng, dst, o[:, qc, 0:HD],
                                      rec[:, qc:qc + 1], None, AluOpType.mult)

                xts = {}

                # ---- post-processing, at 128-query granularity -------------
                # each qc block cascades transpose -> proj -> stats -> apply
                # independently, so the final drain is a short pipelined chain
                # instead of four serial 512-wide stages.
                def transpose_chunk(lc, x_lc, qc):
                    # x (q, c) -> xT (c, q) via XBAR DMA transpose, blocked:
                    # out[c, cc, q] = in[q, cc*128 + c]
                    if qc == 0:
                        xts[lc] = xp.tile([128, NC4, 512], BF16, tag="xT",
                                          name="xT")
                    nc.sync.dma_start_transpose(
                        xts[lc][:, :, qc * 128:(qc + 1) * 128], x_lc[:, qc, :])

                def proj_chunk(lc, qc):
                    sl = slice(lc * 512 + qc * 128, lc * 512 + (qc + 1) * 128)
                    xq = slice(qc * 128, (qc + 1) * 128)
                    xT = xts[lc]
                    for it in range(NC4):
                        ps = pp_tile("psy")
                        for cc in range(NC4):
                            nc.tensor.matmul(ps[:, 0:128],
                                             wp[cc][:, it * 128:(it + 1) * 128],
                                             xT[:, cc, xq],
                                             start=(cc == 0), stop=(cc == NC4 - 1))
                        # xres = (y + bp) + petT (reads PSUM: DVE only)
                        bal.add("dve", scale=0.25)
                        nc.vector.scalar_tensor_tensor(
                            xres[it][:, sl], ps[:, 0:128], bp[it],
                            pet32[it][:, sl], AluOpType.add, AluOpType.add)

                stats = {}

                def ln_stats_chunk(lc, qc):
                    sl = slice(lc * 512 + qc * 128, lc * 512 + (qc + 1) * 128)
                    psum = pp_tile("psum_sum")
                    for c in range(NC4):
                        nc.tensor.matmul(psum[0:1, 0:128], ones_c[:],
                                         xres[c][:, sl],
                                         start=(c == 0), stop=(c == NC4 - 1))
                    psq = pp_tile("psum_sq")
                    for c in range(NC4):
                        xsq = tmp.tile([128, 128], F32R, tag="xsq", name="xsq")
                        nc.gpsimd.tensor_tensor(xsq[:], xres[c][:, sl],
                                                xres[c][:, sl], AluOpType.mult)
                        nc.tensor.matmul(psq[0:1, 0:128], ones_c[:], xsq[:],
                                         start=(c == 0), stop=(c == NC4 - 1))
                    key = (lc, qc)
                    mu = lrows.tile([1, 128], F32R, tag="mu", bufs=NQC + 1,
                                    name=f"mu{lc}_{qc}")
                    ve = lrows.tile([1, 128], F32, tag="ve", bufs=2, name="ve")
                    t0 = lrows.tile([1, 128], F32, tag="t0", bufs=2, name="t0")
                    rstd = lrows.tile([1, 128], F32R, tag="rs", bufs=NQC + 1,
                                      name=f"rs{lc}_{qc}")
                    nc.vector.tensor_scalar(mu[:], psum[0:1, 0:128], 1.0 / C,
                                            None, AluOpType.mult)
                    nc.vector.tensor_tensor(t0[:], mu[:], mu[:], AluOpType.mult)
                    # ve = sumsq/C + eps - mu^2
                    nc.vector.scalar_tensor_tensor(ve[:], psq[0:1, 0:128],
                                                   1.0 / C, t0[:],
                                                   AluOpType.mult,
                                                   AluOpType.subtract)
                    nc.vector.tensor_scalar(ve[:], ve[:], LN_EPS, None,
                                            AluOpType.add)
                    nc.scalar.activation(t0[:], ve[:],
                                         mybir.ActivationFunctionType.Sqrt)
                    r0 = lrows.tile([1, 128], F32, tag="r0", bufs=2, name="r0")
                    nc.vector.reciprocal(r0[:], t0[:])
                    # one Newton step: rstd = r0 * (1.5 - 0.5 * ve * r0^2)
                    nc.vector.tensor_tensor(t0[:], r0[:], r0[:], AluOpType.mult)
                    nc.vector.tensor_tensor(t0[:], t0[:], ve[:], AluOpType.mult)
                    nc.vector.tensor_scalar(t0[:], t0[:], -0.5, 1.5,
                                            AluOpType.mult, AluOpType.add)
                    nc.vector.tensor_tensor(rstd[:], r0[:], t0[:],
                                            AluOpType.mult)
                    stats[key] = (mu, rstd)

                def ln_apply_chunk(lc, qc):
                    sl = slice(lc * 512 + qc * 128, lc * 512 + (qc + 1) * 128)
                    mu, rstd = stats[(lc, qc)]
                    bc = pp_tile("bc")
                    nc.tensor.matmul(bc[:, 0:128], ones_r[:], mu[:])
                    nc.tensor.matmul(bc[:, 128:256], ones_r[:], rstd[:])
                    # stage the broadcast tiles to SBUF (ACT) so the apply
                    # chain can run on GPSIMD, which cannot read PSUM
                    bcs = tmp.tile([128, 256], F32, tag="bcs", bufs=2,
                                   name="bcs")
                    nc.scalar.copy(bcs[:], bc[:, 0:256])
                    bal.add("act", scale=0.5)
                    for c in range(NC4):
                        t = tmp.tile([128, 128], F32, tag="lnt", bufs=4,
                                     name="lnt")
                        y = yout.tile([128, 128], F32, tag="y", bufs=4,
                                      name="yout")
                        nc.gpsimd.tensor_tensor(t[:], xres[c][:, sl],
                                                bcs[:, 0:128],
                                                AluOpType.subtract)
                        nc.gpsimd.tensor_tensor(t[:], t[:], bcs[:, 128:256],
                                                AluOpType.mult)
                        nc.gpsimd.tensor_scalar(y[:], t[:], gamma[c],
                                                beta[c], AluOpType.mult,
                                                AluOpType.add)
                        nc.sync.dma_start(out_d[c * 128:(c + 1) * 128, sl], y[:])

                chunks = []
                for lc in range(NLQ):
                    x_lc = xp.tile([128, NQC, 512], BF16, tag="x", name="x")
                    for pair in range(NH // 2):
                        attention(pair, lc, x_lc)
                        for _ in range(min(4, len(chunks))):
                            chunks.pop(0)()
                    for qc in range(NQC):
                        chunks += [
                            lambda lc=lc, x=x_lc, qc=qc: transpose_chunk(lc, x, qc),
                            lambda lc=lc, qc=qc: proj_chunk(lc, qc),
                            lambda lc=lc, qc=qc: ln_stats_chunk(lc, qc),
                            lambda lc=lc, qc=qc: ln_apply_chunk(lc, qc)]
                while chunks:
                    chunks.pop(0)()

    nc.compile()
    return nc


def prep_core_inputs(inputs):
    """Shard + lay out the full inputs for the 8 cores."""
    pet = np.asarray(inputs["pet_feat"], np.float32).reshape(B, C, L)
    ct = np.asarray(inputs["ct_feat"], np.float32).reshape(B, C, L)
    bf = ml_dtypes.bfloat16
    wq_t = np.ascontiguousarray(np.asarray(inputs["Wq"], np.float32).T).astype(bf)
    wk_t = np.ascontiguousarray(np.asarray(inputs["Wk"], np.float32).T).astype(bf)
    wv_t = np.ascontiguousarray(np.asarray(inputs["Wv"], np.float32).T).astype(bf)
    wp_t = np.ascontiguousarray(np.asarray(inputs["Wp"], np.float32).T).astype(bf)
    gamma = np.asarray(inputs["gamma"], np.float32).reshape(C, 1)
    beta = np.asarray(inputs["beta"], np.float32).reshape(C, 1)
    bp = np.asarray(inputs["bp"], np.float32).reshape(C, 1)
    in_maps = []
    for core in range(NCORES):
        b, j = divmod(core, 4)
        sl = slice(j * LQ, (j + 1) * LQ)
        pet_sl = np.ascontiguousarray(pet[b][:, sl])
        in_maps.append({
            "pet_t_f32": pet_sl,
            "pet_t_bf16": pet_sl.astype(bf),
            "ct_t_bf16": np.ascontiguousarray(ct[b]).astype(bf),
            "wq_t": wq_t, "wk_t": wk_t, "wv_t": wv_t, "wp_t": wp_t,
            "gamma": gamma, "beta": beta, "bp": bp,
        })
    return in_maps


def assemble_output(results):
    out = np.empty((B, C, L), np.float32)
    for core in range(NCORES):
        b, j = divmod(core, 4)
        out[b][:, j * LQ:(j + 1) * LQ] = results[core]["out"]
    return out.reshape(B, C, H, W)


_NC_CACHE = {}


def get_nc(debug=False):
    key = debug
    if key not in _NC_CACHE:
        _NC_CACHE[key] = build_nc(debug=debug)
    return _NC_CACHE[key]


def kernel(**inputs):
    nc = get_nc()
    in_maps = prep_core_inputs(inputs)
    res = bass_utils.run_bass_kernel_spmd(nc, in_maps, list(range(NCORES)))
    return assemble_output(res.results)
